# revision 19
# baseline (speedup 1.0000x reference)
"""Bass/Trainium2 kernel for nn_GatherUpdate: LayerNorm + Linear + per-atom
row gather + residual add, data-parallel over batch across 8 NeuronCores.

reference:
    normed = LayerNorm(s) * gamma + beta            # s: [B, 2048, 384]
    upd    = normed @ W.T                           # W: [128, 384] -> [B, 2048, 128]
    out    = atom_embed + upd[:, cond_to_s_idx, :]  # atom_embed: [B, 32768, 128]

Per-core plan (core b handles batch b). Everything runs in TRANSPOSED space
(c_atom on partitions). The LN+Linear is folded algebraically into matmuls:
upd^T = rstd*(Wg^T s^T) - rstd*mu*wgsum + bias (Wg/wgsum/bias host-folded).

The gather is restructured as a sorted one-hot EXPANSION on the PE. The host
sorts atoms by residue index and lays them out in 16 windows of 2560 output
columns, window w holding atoms whose residue is in [128w, 128w+128). For a
512-column chunk the gather is then one k=128 matmul:
    g[ca, j] = sum_r upd_nat[128w+r, ca] * onehot[r, j]
with onehot[r, j] = (shifted_idx[j] == r) built by one DVE is_equal per chunk
from a host-replicated fp16 shifted-index stream. The atom_embed residual is
accumulated into the same PSUM by an identity matmul, and the beta@W.T bias
rides the ACT PSUM->SBUF drain as a per-partition bias. Window overflow
(>2560 atoms on one 128-residue window; never happens for uniform indices)
falls back to a 512-slot GPSIMD ap_gather cleanup chunk.

This replaces the v1 on-chip ap_gather of all 32768 atoms (27.3 ns/idx ucode
= ~900 us/core measured) with ~35 us of PE work; measured HW total for v1 was
966 us vs 39 ms for the SWDGE row-gather baseline.
"""

import sys

sys.path.insert(0, "/opt/trn_rl_repo")

import numpy as np
import ml_dtypes

B = 8
N_ATOMS = 32768
N_RES = 2048
C_S = 384
C_ATOM = 128
EPS = 1e-5
P = 128
KC = C_S // P  # 3 contraction chunks
QN = 4  # 512-column chunks for stats/matmul/fixup
QS = N_RES // QN  # 512
NW = 16  # residue windows
WR = N_RES // NW  # 128 residues per window
LW = 2304  # output columns per window (2048 mean + 5.8 sigma slack)
CHUNKS = [512, 512, 512, 512, 256]  # per-window column chunks (sum=LW)
CL = 1024  # cleanup (overflow) columns, handled by ap_gather
NCOLS = NW * LW + CL  # 37888
BF16 = ml_dtypes.bfloat16

_compiled = None
_last_aux = None


def _build(repeat=1):
    """Build the per-core program. repeat>1 unrolls the whole pipeline N
    times (used only for timing: wall(N)-wall(1) cancels dispatch/transfer
    overhead)."""
    import concourse.bacc as bacc
    import concourse.tile as tile
    from concourse import mybir
    from concourse.masks import make_identity

    f32 = mybir.dt.float32
    bf16 = mybir.dt.bfloat16
    fp16 = mybir.dt.float16
    i16 = mybir.dt.int16
    AF = mybir.ActivationFunctionType
    OP = mybir.AluOpType

    nc = bacc.Bacc("TRN2", target_bir_lowering=False, debug=False)

    sT_d = nc.dram_tensor("sT", [P, KC * N_RES], bf16, kind="ExternalInput")
    atomS_d = nc.dram_tensor("atomS", [P, NCOLS], bf16, kind="ExternalInput")
    shift_d = nc.dram_tensor("shift", [P, NW * LW], fp16, kind="ExternalInput")
    idxcl_d = nc.dram_tensor("idxcl", [P, CL // 16], i16, kind="ExternalInput")
    wg_d = nc.dram_tensor("wg", [P, KC * C_ATOM], bf16, kind="ExternalInput")
    negw_d = nc.dram_tensor("negw", [1, C_ATOM], bf16, kind="ExternalInput")
    biasc_d = nc.dram_tensor("biasc", [P, 1], f32, kind="ExternalInput")
    iotac_d = nc.dram_tensor("iotac", [P, 1], f32, kind="ExternalInput")
    outS_d = nc.dram_tensor("outS", [P, NCOLS], bf16, kind="ExternalOutput")

    with tile.TileContext(nc) as tc:
        with (
            tc.tile_pool(name="consts", bufs=1) as consts,
            tc.tile_pool(name="sbig", bufs=1) as sbig,
            tc.tile_pool(name="rows", bufs=1) as rows,
            tc.tile_pool(name="t1p", bufs=2) as t1p,
            tc.tile_pool(name="updp", bufs=1) as updp,
            tc.tile_pool(name="ps_stat", bufs=1, space="PSUM") as ps_stat,
            tc.tile_pool(name="ps_a", bufs=1, space="PSUM") as ps_a,
            tc.tile_pool(name="ps_t", bufs=1, space="PSUM") as ps_t,
            tc.tile_pool(name="ps_e", bufs=2, space="PSUM") as ps_e,
            tc.tile_pool(name="ohp", bufs=4) as ohp,
            tc.tile_pool(name="shp", bufs=3) as shp,
            tc.tile_pool(name="atp", bufs=3) as atp,
            tc.tile_pool(name="otp", bufs=3) as otp,
            tc.tile_pool(name="clp", bufs=1) as clp,
        ):
            # --- constants (loaded once) ---
            wg_sb = consts.tile([P, KC * C_ATOM], bf16)
            nc.sync.dma_start(out=wg_sb[:], in_=wg_d.ap())
            negw_sb = consts.tile([1, C_ATOM], bf16)
            nc.sync.dma_start(out=negw_sb[:], in_=negw_d.ap())
            biasc_sb = consts.tile([P, 1], f32)
            nc.sync.dma_start(out=biasc_sb[:], in_=biasc_d.ap())
            iotac = consts.tile([P, 1], f32)
            nc.sync.dma_start(out=iotac[:], in_=iotac_d.ap())
            idxcl_sb = consts.tile([P, CL // 16], i16)
            nc.sync.dma_start(out=idxcl_sb[:], in_=idxcl_d.ap())
            onesc = consts.tile([P, 1], bf16)
            nc.vector.memset(onesc[:], 1.0)
            ones1 = consts.tile([1, P], bf16)
            nc.vector.memset(ones1[:], 1.0)
            eps_t = consts.tile([1, 1], f32)
            nc.vector.memset(eps_t[:], EPS)
            ident32 = consts.tile([P, P], f32)
            make_identity(nc, ident32[:])

            for _rep in range(repeat):
                # --- loads: s first (it gates the critical path) ---
                sT = sbig.tile([P, KC * N_RES], bf16, tag="sT")
                for k in range(KC):
                    nc.sync.dma_start(
                        out=sT[:, k * N_RES : (k + 1) * N_RES],
                        in_=sT_d.ap()[:, k * N_RES : (k + 1) * N_RES],
                    )
                at_cl = clp.tile([P, CL], bf16, tag="atcl")
                nc.sync.dma_start(out=at_cl[:], in_=atomS_d.ap()[:, NW * LW :])

                # --- squares for E[x^2] ---
                sq = sbig.tile([P, KC * N_RES], bf16, tag="sq")
                for k in range(KC):
                    nc.scalar.activation(
                        out=sq[:, k * N_RES : (k + 1) * N_RES],
                        in_=sT[:, k * N_RES : (k + 1) * N_RES],
                        func=AF.Square,
                    )

                # --- LN folded into matmuls: updT = rstd*(Wg^T sT - wgsum x mu)
                mu_sb = rows.tile([1, N_RES], bf16, tag="mu")
                ex2e = rows.tile([1, N_RES], f32, tag="ex2e")
                musq = rows.tile([1, N_RES], f32, tag="musq")
                vare = rows.tile([1, N_RES], f32, tag="vare")
                rstd = rows.tile([1, N_RES], bf16, tag="rstd")
                updT = updp.tile([P, N_RES], f32, tag="updT")

                for q in range(QN):
                    qs = slice(q * QS, (q + 1) * QS)
                    mu_ps = ps_stat.tile([1, QS], f32, tag="mu")
                    sq_ps = ps_stat.tile([1, QS], f32, tag="sq")
                    for k in range(KC):
                        ks = slice(k * N_RES + q * QS, k * N_RES + (q + 1) * QS)
                        nc.tensor.matmul(
                            mu_ps[:],
                            lhsT=onesc[:],
                            rhs=sT[:, ks],
                            start=(k == 0),
                            stop=(k == KC - 1),
                        )
                    for k in range(KC):
                        ks = slice(k * N_RES + q * QS, k * N_RES + (q + 1) * QS)
                        nc.tensor.matmul(
                            sq_ps[:],
                            lhsT=onesc[:],
                            rhs=sq[:, ks],
                            start=(k == 0),
                            stop=(k == KC - 1),
                        )
                    nc.vector.tensor_scalar(
                        out=mu_sb[:, qs],
                        in0=mu_ps[:],
                        scalar1=1.0 / C_S,
                        scalar2=None,
                        op0=OP.mult,
                    )
                    nc.scalar.activation(
                        out=musq[:, qs], in_=mu_ps[:], func=AF.Square, scale=1.0 / C_S
                    )
                    nc.scalar.activation(
                        out=ex2e[:, qs],
                        in_=sq_ps[:],
                        func=AF.Identity,
                        bias=eps_t[:],
                        scale=1.0 / C_S,
                    )
                    nc.vector.scalar_tensor_tensor(
                        out=vare[:, qs],
                        in0=musq[:, qs],
                        scalar=-1.0,
                        in1=ex2e[:, qs],
                        op0=OP.mult,
                        op1=OP.add,
                    )
                    nc.scalar.activation(
                        out=rstd[:, qs], in_=vare[:, qs], func=AF.Abs_reciprocal_sqrt
                    )

                    a_ps = ps_a.tile([P, QS], f32, tag="A")
                    for k in range(KC):
                        ks = slice(k * N_RES + q * QS, k * N_RES + (q + 1) * QS)
                        nc.tensor.matmul(
                            a_ps[:],
                            lhsT=wg_sb[:, k * C_ATOM : (k + 1) * C_ATOM],
                            rhs=sT[:, ks],
                            start=(k == 0),
                            stop=False,
                        )
                    nc.tensor.matmul(
                        a_ps[:],
                        lhsT=negw_sb[:],
                        rhs=mu_sb[:, qs],
                        start=False,
                        stop=True,
                    )
                    b_ps = ps_e.tile([P, 512], f32, tag="e")
                    nc.tensor.matmul(
                        b_ps[:], lhsT=ones1[:], rhs=rstd[:, qs], start=True, stop=True
                    )
                    t1b_sb = t1p.tile([P, QS], f32, tag="t1bsb")
                    nc.scalar.copy(out=t1b_sb[:], in_=b_ps[:])
                    nc.vector.tensor_tensor(
                        out=updT[:, qs], in0=a_ps[:], in1=t1b_sb[:], op=OP.mult
                    )

                # --- overflow cleanup: ap_gather of <=512 leftover atoms ---
                g_cl = clp.tile([P, CL], f32, tag="gcl")
                nc.gpsimd.ap_gather(
                    g_cl[:], updT[:], idxcl_sb[:], P, N_RES, 1, CL
                )
                o_cl = clp.tile([P, CL], bf16, tag="ocl")
                nc.vector.scalar_tensor_tensor(
                    out=o_cl[:],
                    in0=at_cl[:],
                    scalar=biasc_sb[:, 0:1],
                    in1=g_cl[:],
                    op0=OP.add,
                    op1=OP.add,
                )
                nc.sync.dma_start(out=outS_d.ap()[:, NW * LW :], in_=o_cl[:])

                # --- transpose updT into natural-layout bf16 window weights ---
                un_all = updp.tile([P, NW * P], bf16, tag="un")
                for w in range(NW):
                    tr_ps = ps_t.tile([P, P], f32, tag="tr")
                    nc.tensor.transpose(
                        out=tr_ps[:],
                        in_=updT[:, w * WR : (w + 1) * WR],
                        identity=ident32[:],
                    )
                    nc.scalar.copy(out=un_all[:, w * P : (w + 1) * P], in_=tr_ps[:])

                # --- sorted one-hot expansion + residual add, per window ---
                for w in range(NW):
                    sh = shp.tile([P, LW], fp16, tag="sh")
                    nc.sync.dma_start(
                        out=sh[:], in_=shift_d.ap()[:, w * LW : (w + 1) * LW]
                    )
                    at = atp.tile([P, LW], bf16, tag="at")
                    nc.sync.dma_start(
                        out=at[:], in_=atomS_d.ap()[:, w * LW : (w + 1) * LW]
                    )
                    ot = otp.tile([P, LW], bf16, tag="ot")
                    off = 0
                    for ci, ln in enumerate(CHUNKS):
                        cs = slice(off, off + ln)
                        off += ln
                        oh = ohp.tile([P, 512], bf16, tag="oh")
                        nc.vector.tensor_scalar(
                            out=oh[:, :ln],
                            in0=sh[:, cs],
                            scalar1=iotac[:, 0:1],
                            scalar2=None,
                            op0=OP.is_equal,
                        )
                        e_ps = ps_e.tile([P, 512], f32, tag="e")
                        nc.tensor.matmul(
                            e_ps[:, :ln],
                            lhsT=un_all[:, w * P : (w + 1) * P],
                            rhs=oh[:, :ln],
                            start=True,
                            stop=True,
                        )
                        if ci in (1, 3):
                            # unfused path: ACT drains PSUM (+bias), Pool adds
                            g_sb = ohp.tile([P, 512], f32, tag="g")
                            nc.scalar.activation(
                                out=g_sb[:, :ln],
                                in_=e_ps[:, :ln],
                                func=AF.Identity,
                                bias=biasc_sb[:, 0:1],
                            )
                            nc.gpsimd.tensor_tensor(
                                out=ot[:, cs], in0=at[:, cs], in1=g_sb[:, :ln], op=OP.add
                            )
                        else:
                            # fused PSUM drain + residual add + bias (one DVE pass)
                            nc.vector.scalar_tensor_tensor(
                                out=ot[:, cs],
                                in0=at[:, cs],
                                scalar=biasc_sb[:, 0:1],
                                in1=e_ps[:, :ln],
                                op0=OP.add,
                                op1=OP.add,
                            )
                    nc.scalar.dma_start(
                        out=outS_d.ap()[:, w * LW : (w + 1) * LW], in_=ot[:]
                    )

    nc.compile()
    return nc


def _prep_core_inputs(atom_embed, s, cond_to_s_idx, ln_gamma, ln_beta, W):
    """Host-side sharding + layout marshalling: transposes, LN param folding,
    and the sorted-window atom layout."""
    global _last_aux
    wg_full = (W * ln_gamma[None, :]).T.astype(np.float32)  # [C_S, C_ATOM]
    wg_host = np.ascontiguousarray(
        wg_full.reshape(KC, P, C_ATOM).transpose(1, 0, 2).reshape(P, KC * C_ATOM)
    ).astype(BF16)
    negw_host = np.ascontiguousarray(-wg_full.sum(axis=0).reshape(1, C_ATOM)).astype(
        BF16
    )
    biasc_host = np.ascontiguousarray(
        (W.astype(np.float32) @ ln_beta.astype(np.float32)).reshape(P, 1)
    )
    iotac_host = np.arange(P, dtype=np.float32).reshape(P, 1)

    in_maps = []
    aux = []
    for b in range(B):
        sT = np.ascontiguousarray(s[b].T.astype(np.float32))  # [C_S, N_RES]
        sT_host = np.ascontiguousarray(
            sT.reshape(KC, P, N_RES).transpose(1, 0, 2).reshape(P, KC * N_RES)
        ).astype(BF16)

        idxb = np.asarray(cond_to_s_idx[b]).astype(np.int64)  # values < 2048
        order = np.argsort(idxb)
        sidx = idxb[order]
        win = (sidx // WR).astype(np.int64)
        counts = np.bincount(win, minlength=NW)
        starts = np.zeros(NW, np.int64)
        starts[1:] = np.cumsum(counts)[:-1]

        cols = np.full(NCOLS, -1, dtype=np.int64)  # col -> atom id
        shifted = np.zeros(NW * LW, dtype=np.float16)
        clean_atoms, clean_idx = [], []
        for w in range(NW):
            n, st = int(counts[w]), int(starts[w])
            take = min(n, LW)
            cols[w * LW : w * LW + take] = order[st : st + take]
            shifted[w * LW : w * LW + take] = (sidx[st : st + take] - w * WR).astype(
                np.float16
            )
            if n > take:
                clean_atoms.extend(order[st + take : st + n].tolist())
                clean_idx.extend(sidx[st + take : st + n].tolist())
        assert len(clean_atoms) <= CL, (
            f"window overflow {len(clean_atoms)} > {CL}: indices too concentrated"
        )
        npad = CL - len(clean_atoms)
        cols[NW * LW :] = np.array(clean_atoms + [-1] * npad, dtype=np.int64)
        cl_idx = np.array(clean_idx + [0] * npad, dtype=np.int16)
        idxcl_host = np.ascontiguousarray(
            np.tile(np.ascontiguousarray(cl_idx.reshape(CL // 16, 16).T), (P // 16, 1))
        )

        atomT = atom_embed[b].T  # [C_ATOM, N_ATOMS] view
        atomS = np.zeros((P, NCOLS), dtype=np.float32)
        valid = cols >= 0
        atomS[:, valid] = atomT[:, cols[valid]]
        atomS_host = atomS.astype(BF16)

        shift_host = np.ascontiguousarray(
            np.broadcast_to(shifted[None, :], (P, NW * LW))
        )

        in_maps.append(
            {
                "sT": sT_host,
                "atomS": atomS_host,
                "shift": shift_host,
                "idxcl": idxcl_host,
                "wg": wg_host,
                "negw": negw_host,
                "biasc": biasc_host,
                "iotac": iotac_host,
            }
        )
        aux.append((cols, valid))
    _last_aux = aux
    return in_maps


def _gather_output(res):
    out = np.empty((B, N_ATOMS, C_ATOM), dtype=np.float32)
    for b in range(B):
        cols, valid = _last_aux[b]
        outS = res.results[b]["outS"].astype(np.float32)  # [P, NCOLS]
        out[b][cols[valid], :] = outS[:, valid].T
    return out


def kernel(atom_embed, s, cond_to_s_idx, ln_gamma, ln_beta, W):
    global _compiled
    from concourse.bass_utils import run_bass_kernel_spmd

    atom_embed = np.asarray(atom_embed, dtype=np.float32)
    s = np.asarray(s, dtype=np.float32)
    cond_to_s_idx = np.asarray(cond_to_s_idx)
    ln_gamma = np.asarray(ln_gamma, dtype=np.float32)
    ln_beta = np.asarray(ln_beta, dtype=np.float32)
    W = np.asarray(W, dtype=np.float32)

    if _compiled is None:
        _compiled = _build()
    in_maps = _prep_core_inputs(atom_embed, s, cond_to_s_idx, ln_gamma, ln_beta, W)
    res = run_bass_kernel_spmd(_compiled, in_maps, core_ids=list(range(B)))
    return _gather_output(res)


# revision 29
# speedup vs baseline: 1.1135x; 1.1135x over previous
"""Bass/Trainium2 kernel for nn_GatherUpdate: LayerNorm + Linear + per-atom
row gather + residual add, data-parallel over batch across 8 NeuronCores.

reference:
    normed = LayerNorm(s) * gamma + beta            # s: [B, 2048, 384]
    upd    = normed @ W.T                           # W: [128, 384] -> [B, 2048, 128]
    out    = atom_embed + upd[:, cond_to_s_idx, :]  # atom_embed: [B, 32768, 128]

Per-core plan (core b handles batch b). Everything runs in TRANSPOSED space
(c_atom on partitions). LN+Linear folded algebraically into matmuls:
upd^T = rstd*(Wg^T s^T) - rstd*mu*wgsum + bias (Wg/wgsum/bias host-folded);
stats come from ones-column matmuls, the rank-1 mu correction is accumulated
in PSUM by a k=1 matmul, rstd is PE-broadcast and applied by one DVE pass.

The gather runs as a sorted one-hot EXPANSION on the PE: the host sorts atoms
by residue and lays them into 16 windows of LW output columns (window w =
residues [128w, 128w+128)), so a column chunk is one k=128 matmul
    g[ca, j] = sum_r upd_nat[128w+r, ca] * onehot[r, j],
with onehot[r, j] = (shifted_idx[j] == r) built by DVE is_equal from a
host-replicated fp16 shifted-index stream. Drains alternate between a fused
DVE pass (PSUM + atom + bias in one scalar_tensor_tensor) and an unfused
ACT-drain + GPSIMD-add pair to balance engines. Window overflow (uniform
indices never hit it) falls back to a 1024-slot GPSIMD ap_gather cleanup.
Windows 4q..4q+3 are issued right after updT quarter q is finished so the
PE stream interleaves LN chains with expansion matmuls (engine streams are
in-order; issuing all LN first would serialize the phases).

Measured: SWDGE row-gather baseline 39 ms; on-chip ap_gather of all atoms
966 us (27.3 ns/idx ucode); this design 151 us and iterating.
"""

import sys

sys.path.insert(0, "/opt/trn_rl_repo")

import numpy as np
import ml_dtypes

B = 8
N_ATOMS = 32768
N_RES = 2048
C_S = 384
C_ATOM = 128
EPS = 1e-5
P = 128
KC = C_S // P  # 3 contraction chunks
QN = 4  # 512-column chunks for stats/matmul/fixup
QS = N_RES // QN  # 512
NW = 16  # residue windows
WR = N_RES // NW  # 128 residues per window
LW = 2304  # output columns per window (2048 mean + 5.8 sigma slack)
CHUNKS = [1024, 1024, 256]  # per-window column chunks (sum=LW)
CL = 512  # cleanup (overflow) columns, handled by ap_gather
NCOLS = NW * LW + CL
BF16 = ml_dtypes.bfloat16

_compiled = None
_last_aux = None


def _build(repeat=1):
    """Build the per-core program. repeat>1 unrolls the whole pipeline N
    times (used only for timing)."""
    import concourse.bacc as bacc
    import concourse.tile as tile
    from concourse import mybir
    from concourse.masks import make_identity

    f32 = mybir.dt.float32
    bf16 = mybir.dt.bfloat16
    fp16 = mybir.dt.float16
    i16 = mybir.dt.int16
    AF = mybir.ActivationFunctionType
    OP = mybir.AluOpType

    nc = bacc.Bacc("TRN2", target_bir_lowering=False, debug=False)

    sT_d = nc.dram_tensor("sT", [P, KC * N_RES], bf16, kind="ExternalInput")
    atomS_d = nc.dram_tensor("atomS", [P, NCOLS], bf16, kind="ExternalInput")
    shift_d = nc.dram_tensor("shift", [P, NW * LW], fp16, kind="ExternalInput")
    idxcl_d = nc.dram_tensor("idxcl", [P, CL // 16], i16, kind="ExternalInput")
    wg_d = nc.dram_tensor("wg", [P, KC * C_ATOM], bf16, kind="ExternalInput")
    negw_d = nc.dram_tensor("negw", [1, C_ATOM], bf16, kind="ExternalInput")
    biasc_d = nc.dram_tensor("biasc", [P, 1], f32, kind="ExternalInput")
    iotac_d = nc.dram_tensor("iotac", [P, 1], f32, kind="ExternalInput")
    outS_d = nc.dram_tensor("outS", [P, NCOLS], bf16, kind="ExternalOutput")

    with tile.TileContext(nc) as tc:
        with (
            tc.tile_pool(name="consts", bufs=1) as consts,
            tc.tile_pool(name="sbig", bufs=1) as sbig,
            tc.tile_pool(name="rows", bufs=1) as rows,
            tc.tile_pool(name="t1p", bufs=2) as t1p,
            tc.tile_pool(name="updp", bufs=1) as updp,
            tc.tile_pool(name="ps_stat", bufs=1, space="PSUM") as ps_stat,
            tc.tile_pool(name="ps_a", bufs=1, space="PSUM") as ps_a,
            tc.tile_pool(name="ps_t", bufs=1, space="PSUM") as ps_t,
            tc.tile_pool(name="ps_e", bufs=2, space="PSUM") as ps_e,
            tc.tile_pool(name="ohp", bufs=4) as ohp,
            tc.tile_pool(name="shp", bufs=6) as shp,
            tc.tile_pool(name="atp", bufs=6) as atp,
            tc.tile_pool(name="otp", bufs=3) as otp,
            tc.tile_pool(name="clp", bufs=1) as clp,
        ):
            # --- constants (loaded once) ---
            wg_sb = consts.tile([P, KC * C_ATOM], bf16)
            nc.sync.dma_start(out=wg_sb[:], in_=wg_d.ap())
            negw_sb = consts.tile([1, C_ATOM], bf16)
            nc.sync.dma_start(out=negw_sb[:], in_=negw_d.ap())
            biasc_sb = consts.tile([P, 1], f32)
            nc.sync.dma_start(out=biasc_sb[:], in_=biasc_d.ap())
            iotac = consts.tile([P, 1], f32)
            nc.sync.dma_start(out=iotac[:], in_=iotac_d.ap())
            idxcl_sb = consts.tile([P, CL // 16], i16)
            nc.sync.dma_start(out=idxcl_sb[:], in_=idxcl_d.ap())
            ones1 = consts.tile([1, P], bf16)
            nc.vector.memset(ones1[:], 1.0)
            onesc = consts.tile([P, 1], bf16)
            nc.vector.memset(onesc[:], 1.0)
            eps_t = consts.tile([1, 1], f32)
            nc.vector.memset(eps_t[:], EPS)
            ident32 = consts.tile([P, P], f32)
            make_identity(nc, ident32[:])

            for _rep in range(repeat):
                # --- loads: s first (it gates the critical path) ---
                sT = sbig.tile([P, KC * N_RES], bf16, tag="sT")
                for k in range(KC):
                    nc.sync.dma_start(
                        out=sT[:, k * N_RES : (k + 1) * N_RES],
                        in_=sT_d.ap()[:, k * N_RES : (k + 1) * N_RES],
                    )
                at_cl = clp.tile([P, CL], bf16, tag="atcl")
                nc.sync.dma_start(out=at_cl[:], in_=atomS_d.ap()[:, NW * LW :])

                # --- squares for E[x^2] ---
                sq = sbig.tile([P, KC * N_RES], bf16, tag="sq")
                for k in range(KC):
                    nc.scalar.activation(
                        out=sq[:, k * N_RES : (k + 1) * N_RES],
                        in_=sT[:, k * N_RES : (k + 1) * N_RES],
                        func=AF.Square,
                    )

                mu_sb = rows.tile([1, N_RES], bf16, tag="mu")
                rstd = rows.tile([1, N_RES], bf16, tag="rstd")
                updT = updp.tile([P, N_RES], f32, tag="updT")
                un_all = updp.tile([P, NW * P], bf16, tag="un")

                for q in range(QN):
                    qs = slice(q * QS, (q + 1) * QS)
                    # --- LN chain for this 512-residue quarter ---
                    ex2e = t1p.tile([1, QS], f32, tag="ex2e")
                    musq = t1p.tile([1, QS], f32, tag="musq")
                    vare = t1p.tile([1, QS], f32, tag="vare")
                    mu_ps = ps_stat.tile([1, QS], f32, tag="mu")
                    sq_ps = ps_stat.tile([1, QS], f32, tag="sq")
                    for k in range(KC):
                        ks = slice(k * N_RES + q * QS, k * N_RES + (q + 1) * QS)
                        nc.tensor.matmul(
                            mu_ps[:],
                            lhsT=onesc[:],
                            rhs=sT[:, ks],
                            start=(k == 0),
                            stop=(k == KC - 1),
                        )
                    for k in range(KC):
                        ks = slice(k * N_RES + q * QS, k * N_RES + (q + 1) * QS)
                        nc.tensor.matmul(
                            sq_ps[:],
                            lhsT=onesc[:],
                            rhs=sq[:, ks],
                            start=(k == 0),
                            stop=(k == KC - 1),
                        )
                    nc.vector.tensor_scalar(
                        out=mu_sb[:, qs],
                        in0=mu_ps[:],
                        scalar1=1.0 / C_S,
                        scalar2=None,
                        op0=OP.mult,
                    )
                    nc.scalar.activation(
                        out=musq[:], in_=mu_ps[:], func=AF.Square, scale=1.0 / C_S
                    )
                    nc.scalar.activation(
                        out=ex2e[:],
                        in_=sq_ps[:],
                        func=AF.Identity,
                        bias=eps_t[:],
                        scale=1.0 / C_S,
                    )
                    nc.vector.scalar_tensor_tensor(
                        out=vare[:],
                        in0=musq[:],
                        scalar=-1.0,
                        in1=ex2e[:],
                        op0=OP.mult,
                        op1=OP.add,
                    )
                    nc.scalar.activation(
                        out=rstd[:, qs], in_=vare[:], func=AF.Abs_reciprocal_sqrt
                    )

                    a_ps = ps_a.tile([P, QS], f32, tag="A")
                    for k in range(KC):
                        ks = slice(k * N_RES + q * QS, k * N_RES + (q + 1) * QS)
                        nc.tensor.matmul(
                            a_ps[:],
                            lhsT=wg_sb[:, k * C_ATOM : (k + 1) * C_ATOM],
                            rhs=sT[:, ks],
                            start=(k == 0),
                            stop=False,
                        )
                    nc.tensor.matmul(
                        a_ps[:],
                        lhsT=negw_sb[:],
                        rhs=mu_sb[:, qs],
                        start=False,
                        stop=True,
                    )
                    b_ps = ps_e.tile([P, 1024], f32, tag="e")
                    nc.tensor.matmul(
                        b_ps[:, :QS], lhsT=ones1[:], rhs=rstd[:, qs], start=True, stop=True
                    )
                    t1b_sb = t1p.tile([P, QS], f32, tag="t1bsb")
                    nc.scalar.copy(out=t1b_sb[:], in_=b_ps[:, :QS])
                    nc.vector.tensor_tensor(
                        out=updT[:, qs], in0=a_ps[:], in1=t1b_sb[:], op=OP.mult
                    )

                    if q == QN - 1:
                        # --- overflow cleanup: ap_gather of leftover atoms ---
                        g_cl = clp.tile([P, CL], f32, tag="gcl")
                        nc.gpsimd.ap_gather(
                            g_cl[:], updT[:], idxcl_sb[:], P, N_RES, 1, CL
                        )
                        o_cl = clp.tile([P, CL], bf16, tag="ocl")
                        nc.vector.scalar_tensor_tensor(
                            out=o_cl[:],
                            in0=at_cl[:],
                            scalar=biasc_sb[:, 0:1],
                            in1=g_cl[:],
                            op0=OP.add,
                            op1=OP.add,
                        )
                        nc.sync.dma_start(out=outS_d.ap()[:, NW * LW :], in_=o_cl[:])

                    # --- windows 4q..4q+3 (depend only on updT[:, qs]) ---
                    for w in range(4 * q, 4 * q + 4):
                        tr_ps = ps_t.tile([P, P], f32, tag="tr")
                        nc.tensor.transpose(
                            out=tr_ps[:],
                            in_=updT[:, w * WR : (w + 1) * WR],
                            identity=ident32[:],
                        )
                        nc.scalar.copy(
                            out=un_all[:, w * P : (w + 1) * P], in_=tr_ps[:]
                        )
                        sh = shp.tile([P, LW], fp16, tag="sh")
                        nc.scalar.dma_start(
                            out=sh[:], in_=shift_d.ap()[:, w * LW : (w + 1) * LW]
                        )
                        at = atp.tile([P, LW], bf16, tag="at")
                        nc.sync.dma_start(
                            out=at[:], in_=atomS_d.ap()[:, w * LW : (w + 1) * LW]
                        )
                        ot = otp.tile([P, LW], bf16, tag="ot")
                        off = 0
                        for ci, ln in enumerate(CHUNKS):
                            cs = slice(off, off + ln)
                            off += ln
                            oh = ohp.tile([P, 1024], bf16, tag="oh")
                            nc.vector.tensor_scalar(
                                out=oh[:, :ln],
                                in0=sh[:, cs],
                                scalar1=iotac[:, 0:1],
                                scalar2=None,
                                op0=OP.is_equal,
                            )
                            # bank-aligned matmuls fill one 1024-wide PSUM tile
                            e_ps = ps_e.tile([P, 1024], f32, tag="e")
                            for h in range(0, ln, 512):
                                hl = min(512, ln - h)
                                nc.tensor.matmul(
                                    e_ps[:, h : h + hl],
                                    lhsT=un_all[:, w * P : (w + 1) * P],
                                    rhs=oh[:, h : h + hl],
                                    start=True,
                                    stop=True,
                                )
                            if ci == 1 and q < QN - 1:
                                # unfused: ACT drains PSUM (+bias), Pool adds
                                g_sb = ohp.tile([P, 1024], f32, tag="g")
                                nc.scalar.activation(
                                    out=g_sb[:, :ln],
                                    in_=e_ps[:, :ln],
                                    func=AF.Identity,
                                    bias=biasc_sb[:, 0:1],
                                )
                                nc.gpsimd.tensor_tensor(
                                    out=ot[:, cs],
                                    in0=at[:, cs],
                                    in1=g_sb[:, :ln],
                                    op=OP.add,
                                )
                            else:
                                # fused PSUM drain + residual add + bias (DVE)
                                nc.vector.scalar_tensor_tensor(
                                    out=ot[:, cs],
                                    in0=at[:, cs],
                                    scalar=biasc_sb[:, 0:1],
                                    in1=e_ps[:, :ln],
                                    op0=OP.add,
                                    op1=OP.add,
                                )
                        nc.scalar.dma_start(
                            out=outS_d.ap()[:, w * LW : (w + 1) * LW], in_=ot[:]
                        )

    nc.compile()
    return nc


def _prep_core_inputs(atom_embed, s, cond_to_s_idx, ln_gamma, ln_beta, W):
    """Host-side sharding + layout marshalling: transposes, LN param folding,
    and the sorted-window atom layout."""
    global _last_aux
    wg_full = (W * ln_gamma[None, :]).T.astype(np.float32)  # [C_S, C_ATOM]
    wg_host = np.ascontiguousarray(
        wg_full.reshape(KC, P, C_ATOM).transpose(1, 0, 2).reshape(P, KC * C_ATOM)
    ).astype(BF16)
    negw_host = np.ascontiguousarray(-wg_full.sum(axis=0).reshape(1, C_ATOM)).astype(
        BF16
    )
    biasc_host = np.ascontiguousarray(
        (W.astype(np.float32) @ ln_beta.astype(np.float32)).reshape(P, 1)
    )
    iotac_host = np.arange(P, dtype=np.float32).reshape(P, 1)

    in_maps = []
    aux = []
    for b in range(B):
        sT = np.ascontiguousarray(s[b].T.astype(np.float32))  # [C_S, N_RES]
        sT_host = np.ascontiguousarray(
            sT.reshape(KC, P, N_RES).transpose(1, 0, 2).reshape(P, KC * N_RES)
        ).astype(BF16)

        idxb = np.asarray(cond_to_s_idx[b]).astype(np.int64)  # values < 2048
        order = np.argsort(idxb)
        sidx = idxb[order]
        win = (sidx // WR).astype(np.int64)
        counts = np.bincount(win, minlength=NW)
        starts = np.zeros(NW, np.int64)
        starts[1:] = np.cumsum(counts)[:-1]

        cols = np.full(NCOLS, -1, dtype=np.int64)  # col -> atom id
        shifted = np.zeros(NW * LW, dtype=np.float16)
        clean_atoms, clean_idx = [], []
        for w in range(NW):
            n, st = int(counts[w]), int(starts[w])
            take = min(n, LW)
            cols[w * LW : w * LW + take] = order[st : st + take]
            shifted[w * LW : w * LW + take] = (sidx[st : st + take] - w * WR).astype(
                np.float16
            )
            if n > take:
                clean_atoms.extend(order[st + take : st + n].tolist())
                clean_idx.extend(sidx[st + take : st + n].tolist())
        assert len(clean_atoms) <= CL, (
            f"window overflow {len(clean_atoms)} > {CL}: indices too concentrated"
        )
        npad = CL - len(clean_atoms)
        cols[NW * LW :] = np.array(clean_atoms + [-1] * npad, dtype=np.int64)
        cl_idx = np.array(clean_idx + [0] * npad, dtype=np.int16)
        idxcl_host = np.ascontiguousarray(
            np.tile(np.ascontiguousarray(cl_idx.reshape(CL // 16, 16).T), (P // 16, 1))
        )

        atomT = atom_embed[b].T  # [C_ATOM, N_ATOMS] view
        atomS = np.zeros((P, NCOLS), dtype=np.float32)
        valid = cols >= 0
        atomS[:, valid] = atomT[:, cols[valid]]
        atomS_host = atomS.astype(BF16)

        shift_host = np.ascontiguousarray(
            np.broadcast_to(shifted[None, :], (P, NW * LW))
        )

        in_maps.append(
            {
                "sT": sT_host,
                "atomS": atomS_host,
                "shift": shift_host,
                "idxcl": idxcl_host,
                "wg": wg_host,
                "negw": negw_host,
                "biasc": biasc_host,
                "iotac": iotac_host,
            }
        )
        aux.append((cols, valid))
    _last_aux = aux
    return in_maps


def _gather_output(res):
    out = np.empty((B, N_ATOMS, C_ATOM), dtype=np.float32)
    for b in range(B):
        cols, valid = _last_aux[b]
        outS = res.results[b]["outS"].astype(np.float32)  # [P, NCOLS]
        out[b][cols[valid], :] = outS[:, valid].T
    return out


def kernel(atom_embed, s, cond_to_s_idx, ln_gamma, ln_beta, W):
    global _compiled
    from concourse.bass_utils import run_bass_kernel_spmd

    atom_embed = np.asarray(atom_embed, dtype=np.float32)
    s = np.asarray(s, dtype=np.float32)
    cond_to_s_idx = np.asarray(cond_to_s_idx)
    ln_gamma = np.asarray(ln_gamma, dtype=np.float32)
    ln_beta = np.asarray(ln_beta, dtype=np.float32)
    W = np.asarray(W, dtype=np.float32)

    if _compiled is None:
        _compiled = _build()
    in_maps = _prep_core_inputs(atom_embed, s, cond_to_s_idx, ln_gamma, ln_beta, W)
    res = run_bass_kernel_spmd(_compiled, in_maps, core_ids=list(range(B)))
    return _gather_output(res)
